# revision 15
# baseline (speedup 1.0000x reference)
"""Trainium2 Bass kernel for nn_MultiHeadLegalAttention (B=4, S=2048, D=1024, H=4).

Sharding: 8 cores = (batch b in 0..4) x (query-half in 0..2). Each core
computes the 4-head "legal" attention for its 1024 query rows against the
full 2048-key context of its batch, plus the output projection, residual
and layernorm for those rows. Per-head exp-score tiles are shipped raw
(f16) and the host normalizes them into average_weights.

Device layout (per core, [partition, free]):
  xT      [D, S]   embeddings[b].T                (rhs for K/V projections)
  qT/kT   [e, s]   per-head projected Q/K, transposed (e on partitions)
  scoresT [j, i]   key j on partitions, query i on free axis
  softmax denominator via ones-matmul over partitions; attention output
  attT[e, i] accumulated in PSUM over key chunks, scaled by 1/colsum,
  concatenated to concatT[c, i]; out[i, f] = concatT.T @ Wo_w.T + residual.

Matmul dtypes: float32r (full fp32 operands, relaxed multiply, ~1.5e-4)
for the score path; f16 for the post-softmax path (P in [0, e^~6]).
"""

import numpy as np

import concourse.mybir as mybir
from concourse import bacc
from concourse.tile import TileContext
from concourse.bass_utils import run_bass_kernel_spmd

B, S, D, H, HD = 4, 2048, 1024, 4, 256
SQ = S // 2           # queries per core
NKC = S // 128        # key chunks of 128
NQ2 = SQ // 512       # 512-wide free-dim slices of the query range
LN_EPS = 1e-5

F32 = mybir.dt.float32
F32R = mybir.dt.float32r
F16 = mybir.dt.float16
ALU = mybir.AluOpType
ACTF = mybir.ActivationFunctionType

_CACHE = {}


def _build():
    nc = bacc.Bacc("TRN2", target_bir_lowering=False, debug=False)

    # xT columns are host-permuted so this core's 1024 query columns come
    # first; key chunk kc covers permuted positions [kc*128, kc*128+128) and
    # all j-indexed side inputs (jv/mj/vj/bias2) use the same permutation.
    d_xT = nc.dram_tensor("xT", [D, S], F32R, kind="ExternalInput")
    d_wqT = nc.dram_tensor("wqT", [H, HD, HD], F32R, kind="ExternalInput")
    d_wkT = nc.dram_tensor("wkT", [H, HD, HD], F32R, kind="ExternalInput")
    d_wvT = nc.dram_tensor("wvT", [H, HD, HD], F32R, kind="ExternalInput")
    d_b2 = nc.dram_tensor("bias2", [NKC, 128, SQ], F16, kind="ExternalInput")
    d_ibc = nc.dram_tensor("ibc", [128, SQ], F16, kind="ExternalInput")
    d_mibc = nc.dram_tensor("mibc", [128, SQ], F32, kind="ExternalInput")
    d_vibc = nc.dram_tensor("vibc", [128, SQ], F32, kind="ExternalInput")
    d_jv = nc.dram_tensor("jv", [128, NKC], F32, kind="ExternalInput")
    d_mj = nc.dram_tensor("mj", [128, NKC], F32, kind="ExternalInput")
    d_vj = nc.dram_tensor("vj", [128, NKC], F32, kind="ExternalInput")
    d_thr = nc.dram_tensor("thr", [128, 1], F32, kind="ExternalInput")
    d_bsc = nc.dram_tensor("bsc", [128, H], F32, kind="ExternalInput")
    d_emb = nc.dram_tensor("embr", [SQ, D], F32, kind="ExternalInput")
    d_woT = nc.dram_tensor("woT", [D, D], F16, kind="ExternalInput")

    d_out = nc.dram_tensor("out", [SQ, D], F32, kind="ExternalOutput")
    d_expw = nc.dram_tensor("expw", [H, NKC, 128, SQ], F16, kind="ExternalOutput")

    with TileContext(nc) as tc:
        with (
            tc.tile_pool(name="const", bufs=1) as pc,
            tc.tile_pool(name="wts", bufs=2) as pw,
            tc.tile_pool(name="xin", bufs=2) as px,
            tc.tile_pool(name="pq", bufs=1) as pq,
            tc.tile_pool(name="pkv", bufs=2) as pkv,
            tc.tile_pool(name="pexp", bufs=5) as pexp,
            tc.tile_pool(name="pmask", bufs=3) as pmask,
            tc.tile_pool(name="big", bufs=1) as pbig,
            tc.tile_pool(name="pb2", bufs=2) as pb2,
            tc.tile_pool(name="psb", bufs=4) as psb,
            tc.tile_pool(name="prr", bufs=2) as prr,
            tc.tile_pool(name="fin", bufs=2) as pfin,
            tc.tile_pool(name="ps_s", bufs=3, space="PSUM") as ps_s,
            tc.tile_pool(name="ps_a", bufs=1, space="PSUM") as ps_a,
            tc.tile_pool(name="ps_c", bufs=1, space="PSUM") as ps_c,
        ):
            def emit_kv_dma(h):
                """input DMAs for head h's K/V projections."""
                t_wk = pw.tile([128, 2, HD], F32R, tag="wk", name="wk")
                nc.sync.dma_start(
                    out=t_wk[:], in_=d_wkT.ap()[h].rearrange("(dc p) e -> p dc e", p=128)
                )
                t_wv = pw.tile([128, 2, HD], F32R, tag="wv", name="wv")
                nc.sync.dma_start(
                    out=t_wv[:], in_=d_wvT.ap()[h].rearrange("(dc p) e -> p dc e", p=128)
                )
                t_x = []
                for dc in range(2):
                    tx = px.tile([128, S], F32R, tag="x", name="x")
                    nc.sync.dma_start(
                        out=tx[:],
                        in_=d_xT.ap()[(2 * h + dc) * 128:(2 * h + dc + 1) * 128, :],
                    )
                    t_x.append(tx)
                return t_x, t_wk, t_wv

            def emit_kv_mm(h, t_x, t_wk, t_wv):
                t_kT = pkv.tile([128, 2, S], F32R, tag="kT", name="kT")
                for ec in range(2):
                    for nk in range(4):
                        psk = ps_s.tile([128, 512], F32, tag="pss", name="psk")
                        for dc in range(2):
                            nc.tensor.matmul(
                                psk[:],
                                t_wk[:, dc, ec * 128:(ec + 1) * 128],
                                t_x[dc][:, nk * 512:(nk + 1) * 512],
                                start=(dc == 0), stop=(dc == 1),
                            )
                        nc.scalar.copy(t_kT[:, ec, nk * 512:(nk + 1) * 512], psk[:])
                t_v = pkv.tile([128, NKC, HD], F16, tag="v", name="v")
                for kc in range(NKC):
                    psv = ps_s.tile([128, 512], F32, tag="pss", name="psv")
                    for dc in range(2):
                        nc.tensor.matmul(
                            psv[:, 0:HD],
                            t_x[dc][:, kc * 128:(kc + 1) * 128],
                            t_wv[:, dc, :],
                            start=(dc == 0), stop=(dc == 1),
                        )
                    nc.scalar.copy(t_v[:, kc, :], psv[:, 0:HD])
                return t_kT, t_v

            def emit_q(h, t_x):
                """Q^T for head h (query columns are xT cols [0, SQ))."""
                t_wq = pw.tile([128, 2, HD], F32R, tag="wq", name="wq")
                nc.sync.dma_start(
                    out=t_wq[:], in_=d_wqT.ap()[h].rearrange("(dc p) e -> p dc e", p=128)
                )
                t_qT = pq.tile([128, 2, SQ], F32R, tag="qT", name="qT")
                for ec in range(2):
                    for nq in range(NQ2):
                        psq = ps_s.tile([128, 512], F32, tag="pss", name="psq")
                        for dc in range(2):
                            nc.tensor.matmul(
                                psq[:],
                                t_wq[:, dc, ec * 128:(ec + 1) * 128],
                                t_x[dc][:, nq * 512:(nq + 1) * 512],
                                start=(dc == 0), stop=(dc == 1),
                            )
                        nc.scalar.copy(t_qT[:, ec, nq * 512:(nq + 1) * 512], psq[:])
                return t_qT

            # ================= heads =================
            # head-0 input DMAs go first so the PE can start ASAP
            _kv = emit_kv_dma(0)
            t_x = _kv[0]

            # --- constants (small DMAs, queued behind head-0 inputs) ---
            t_ibc = pc.tile([128, SQ], F16)
            nc.sync.dma_start(out=t_ibc[:], in_=d_ibc.ap())
            t_mibc = pc.tile([128, SQ], F32)
            nc.sync.dma_start(out=t_mibc[:], in_=d_mibc.ap())
            t_vibc = pc.tile([128, SQ], F32)
            nc.sync.dma_start(out=t_vibc[:], in_=d_vibc.ap())
            t_jv = pc.tile([128, NKC], F32)
            nc.sync.dma_start(out=t_jv[:], in_=d_jv.ap())
            t_mj = pc.tile([128, NKC], F32)
            nc.sync.dma_start(out=t_mj[:], in_=d_mj.ap())
            t_vj = pc.tile([128, NKC], F32)
            nc.sync.dma_start(out=t_vj[:], in_=d_vj.ap())
            t_thr = pc.tile([128, 1], F32)
            nc.sync.dma_start(out=t_thr[:], in_=d_thr.ap())
            t_bsc = pc.tile([128, H], F32)
            nc.sync.dma_start(out=t_bsc[:], in_=d_bsc.ap())
            t_ones = pc.tile([128, 2], F16)
            nc.vector.memset(t_ones[:], 1.0)
            t_eps = pc.tile([128, 1], F32)
            nc.vector.memset(t_eps[:], LN_EPS)
            t_wo = pbig.tile([128, 8, D], F16, tag="wo")
            for cc in range(8):
                nc.sync.dma_start(
                    out=t_wo[:, cc, :], in_=d_woT.ap()[cc * 128:(cc + 1) * 128, :]
                )
            t_cat = pbig.tile([128, 8, SQ], F16, tag="cat")

            t_kT, t_v = emit_kv_mm(0, *_kv)
            t_qT = emit_q(0, t_x)
            nxt = None
            for h in range(H):
                att = [ps_a.tile([128, 512], F32, tag=f"att{i}", name=f"att{i}") for i in range(4)]
                csum_all = ps_c.tile([34, 512], F32, tag="csa", name="csa")

                def emit_score_chunk(kc):
                    pss = [ps_s.tile([128, 512], F32, tag="pss", name=f"pss{i}") for i in range(NQ2)]
                    for ec in range(2):
                        for nq in range(NQ2):
                            nc.tensor.matmul(
                                pss[nq][:],
                                t_kT[:, ec, kc * 128:(kc + 1) * 128],
                                t_qT[:, ec, nq * 512:(nq + 1) * 512],
                                start=(ec == 0), stop=(ec == 1),
                            )
                    # bias, on [j, i] tiles: j = perm[kc*128 + p], i = q0 + col
                    if h == 0:
                        msk = pmask.tile([128, SQ], F16, tag="msk", name="msk")
                        nc.vector.tensor_scalar(
                            out=msk[:], in0=t_ibc[:],
                            scalar1=t_jv[:, kc:kc + 1], scalar2=None, op0=ALU.is_lt,
                        )
                    elif h == 1:
                        msk = pmask.tile([128, SQ], F16, tag="msk", name="msk")
                        nc.vector.tensor_scalar(
                            out=msk[:], in0=t_mibc[:],
                            scalar1=t_mj[:, kc:kc + 1], scalar2=t_thr[:],
                            op0=ALU.mult, op1=ALU.is_gt,
                        )
                    elif h == 3:
                        msk = pmask.tile([128, SQ], F16, tag="msk", name="msk")
                        nc.vector.tensor_scalar(
                            out=msk[:], in0=t_vibc[:],
                            scalar1=t_vj[:, kc:kc + 1], scalar2=0.5,
                            op0=ALU.mult, op1=ALU.is_gt,
                        )
                    else:  # h == 2: temporal proximity, streamed from DRAM
                        msk = pb2.tile([128, SQ], F16, tag="b2", name="b2")
                        nc.sync.dma_start(out=msk[:], in_=d_b2.ap()[kc])

                    t_e = pexp.tile([128, SQ], F16, tag="expP", name="expP")
                    for nq in range(NQ2):
                        sl = slice(nq * 512, (nq + 1) * 512)
                        t_sb = psb.tile([128, 512], F32, tag="sb", name="sb")
                        if h == 2:
                            nc.vector.tensor_tensor(
                                out=t_sb[:], in0=pss[nq][:], in1=msk[:, sl],
                                op=ALU.add,
                            )
                        else:
                            nc.vector.scalar_tensor_tensor(
                                out=t_sb[:], in0=msk[:, sl],
                                scalar=t_bsc[:, h:h + 1], in1=pss[nq][:],
                                op0=ALU.mult, op1=ALU.add,
                            )
                        nc.scalar.activation(out=t_e[:, sl], in_=t_sb[:], func=ACTF.Exp)
                    return t_e

                def emit_consume_chunk(kc, t_e):
                    for nq in range(NQ2):
                        nc.tensor.matmul(
                            csum_all[nq * 32:nq * 32 + 2, :], t_ones[:],
                            t_e[:, nq * 512:(nq + 1) * 512],
                            start=(kc == 0), stop=(kc == NKC - 1),
                        )
                    for ec in range(2):
                        for nq in range(NQ2):
                            nc.tensor.matmul(
                                att[2 * ec + nq][:],
                                t_v[:, kc, ec * 128:(ec + 1) * 128],
                                t_e[:, nq * 512:(nq + 1) * 512],
                                start=(kc == 0), stop=(kc == NKC - 1),
                            )
                    nc.sync.dma_start(out=d_expw.ap()[h, kc], in_=t_e[:])

                pending = []
                for kc in range(NKC):
                    t_e = emit_score_chunk(kc)
                    pending.append((kc, t_e))
                    if len(pending) > 1:
                        emit_consume_chunk(*pending.pop(0))
                    if kc == 0 and h + 1 < H:
                        nxt_in = emit_kv_dma(h + 1)
                while pending:
                    emit_consume_chunk(*pending.pop(0))

                # ---- normalize attT into concatT ----
                t_cs = prr.tile([1, SQ], F32, tag="cs")
                for nq in range(NQ2):
                    nc.scalar.copy(t_cs[0:1, nq * 512:(nq + 1) * 512],
                                   csum_all[nq * 32:nq * 32 + 1, :])
                # reciprocal is iterative (8 cyc/elem): spread the 1024 sums
                # across all 128 partitions via DMA reshape, recip, reshape back
                t_r8 = prr.tile([128, 8], F32, tag="r8")
                nc.gpsimd.dma_start(out=t_r8[:], in_=t_cs[0:1, :])
                nc.vector.reciprocal(t_r8[:], t_r8[:])
                t_cr = prr.tile([1, SQ], F32, tag="cr")
                nc.gpsimd.dma_start(out=t_cr[0:1, :], in_=t_r8[:])
                t_rbc = prr.tile([128, SQ], F32, tag="rbc")
                nc.gpsimd.partition_broadcast(t_rbc[:], t_cr[0:1, :])
                for ec in range(2):
                    for nq in range(NQ2):
                        sl = slice(nq * 512, (nq + 1) * 512)
                        nc.vector.tensor_tensor(
                            out=t_cat[:, 2 * h + ec, sl],
                            in0=att[2 * ec + nq][:], in1=t_rbc[:, sl], op=ALU.mult,
                        )
                if h + 1 < H:
                    t_kT, t_v = emit_kv_mm(h + 1, *nxt_in)
                    t_qT = emit_q(h + 1, nxt_in[0])

            # ================= output projection + layernorm =================
            for m in range(8):
                t_emb = pfin.tile([128, D], F32, tag="emb")
                nc.sync.dma_start(out=t_emb[:], in_=d_emb.ap()[m * 128:(m + 1) * 128, :])
                t_y = pfin.tile([128, D], F32, tag="y")
                for nf in range(2):
                    pso = ps_s.tile([128, 512], F32, tag="pss", name="pso")
                    for cc in range(8):
                        nc.tensor.matmul(
                            pso[:],
                            t_cat[:, cc, m * 128:(m + 1) * 128],
                            t_wo[:, cc, nf * 512:(nf + 1) * 512],
                            start=(cc == 0), stop=(cc == 7),
                        )
                    nc.vector.tensor_tensor(
                        out=t_y[:, nf * 512:(nf + 1) * 512], in0=pso[:],
                        in1=t_emb[:, nf * 512:(nf + 1) * 512], op=ALU.add,
                    )
                t_st = pfin.tile([128, 2, 6], F32, tag="st")
                for g in range(2):
                    nc.vector.bn_stats(out=t_st[:, g, :], in_=t_y[:, g * 512:(g + 1) * 512])
                t_mv = pfin.tile([128, 2], F32, tag="mv")
                nc.vector.bn_aggr(out=t_mv[:], in_=t_st[:])
                t_sd = pfin.tile([128, 1], F32, tag="sd")
                nc.scalar.activation(
                    out=t_sd[:], in_=t_mv[:, 1:2], func=ACTF.Sqrt, bias=t_eps[:], scale=1.0
                )
                nc.vector.reciprocal(t_sd[:], t_sd[:])
                t_ms = pfin.tile([128, 1], F32, tag="ms")
                nc.vector.tensor_scalar(
                    out=t_ms[:], in0=t_mv[:, 0:1], scalar1=t_sd[:], scalar2=None,
                    op0=ALU.mult,
                )
                t_o = pfin.tile([128, D], F32, tag="o")
                nc.vector.tensor_scalar(
                    out=t_o[:], in0=t_y[:], scalar1=t_sd[:], scalar2=t_ms[:],
                    op0=ALU.mult, op1=ALU.subtract,
                )
                nc.sync.dma_start(out=d_out.ap()[m * 128:(m + 1) * 128, :], in_=t_o[:])

    nc.compile()
    return nc


def _get_nc():
    if "nc" not in _CACHE:
        _CACHE["nc"] = _build()
    return _CACHE["nc"]


def _prep_core_inputs(emb, Wq, Wk, Wv, bias_scalars, Wo_w, Wo_b, ln_g, ln_b):
    """Host-side prep: returns list of 8 per-core input dicts."""
    scale = np.float32(1.0 / np.sqrt(HD))
    wqT = np.ascontiguousarray((Wq * scale).transpose(0, 2, 1)).astype(np.float32)
    wkT = np.ascontiguousarray(Wk.transpose(0, 2, 1)).astype(np.float32)
    wvT = np.ascontiguousarray(Wv.transpose(0, 2, 1)).astype(np.float32)
    woT = np.ascontiguousarray(Wo_w.T).astype(np.float16)

    mag1 = np.sqrt((emb[:, :, HD:2 * HD].astype(np.float32) ** 2).sum(-1))  # [B, S]
    ssum = mag1.astype(np.float64).sum(axis=1)
    thr_v = np.float32((ssum ** 2).sum() / (B * S * S))
    v3 = emb[:, :, D - 1].astype(np.float32)  # [B, S]

    thr = np.full((128, 1), thr_v, np.float32)
    bsc = np.ascontiguousarray(
        np.broadcast_to(np.asarray(bias_scalars, np.float32), (128, H))
    )

    ins = []
    for c in range(8):
        b, half = divmod(c, 2)
        q0 = half * SQ
        xb = emb[b]
        # key-column permutation: this core's query block first
        perm = (np.arange(S) + q0) % S
        xT = np.ascontiguousarray(xb.T.astype(np.float32)[:, perm])
        iv = (q0 + np.arange(SQ, dtype=np.float32))
        ibc = np.ascontiguousarray(np.broadcast_to(iv.astype(np.float16), (128, SQ)))
        mibc = np.ascontiguousarray(np.broadcast_to(mag1[b, q0:q0 + SQ], (128, SQ)))
        vibc = np.ascontiguousarray(np.broadcast_to(v3[b, q0:q0 + SQ], (128, SQ)))
        jvp = perm.astype(np.float32)
        jv_c = np.ascontiguousarray(jvp.reshape(NKC, 128).T)
        mj = np.ascontiguousarray(mag1[b][perm].reshape(NKC, 128).T)
        vj = np.ascontiguousarray(v3[b][perm].reshape(NKC, 128).T)
        jj = perm.astype(np.float64)[:, None]
        ii = (q0 + np.arange(SQ, dtype=np.float64))[None, :]
        b2 = (np.float64(bias_scalars[2]) * np.exp(-np.abs(ii - jj) / S)).astype(
            np.float16
        ).reshape(NKC, 128, SQ)
        embr = (xb[q0:q0 + SQ, :] + np.asarray(Wo_b, np.float32)[None, :]).astype(
            np.float32
        )
        ins.append({
            "xT": xT, "wqT": wqT, "wkT": wkT, "wvT": wvT,
            "bias2": np.ascontiguousarray(b2), "ibc": ibc, "mibc": mibc,
            "vibc": vibc, "jv": jv_c, "mj": mj, "vj": vj, "thr": thr, "bsc": bsc,
            "embr": embr, "woT": woT,
        })
    return ins


def _postprocess(results, ln_g, ln_b):
    out_full = np.empty((B, S, D), np.float32)
    avg = np.empty((B, S, S), np.float32)
    for c in range(8):
        b, half = divmod(c, 2)
        q0 = half * SQ
        out_full[b, q0:q0 + SQ] = results[c]["out"]
        ew = results[c]["expw"].reshape(H, S, SQ).astype(np.float32)  # [h, j_perm, i]
        cs = ew.sum(axis=1)  # [h, i]
        w = np.einsum("hji,hi->ji", ew, (1.0 / cs).astype(np.float32))
        w = np.roll(w, q0, axis=0)  # undo key-column permutation
        avg[b, q0:q0 + SQ, :] = 0.25 * w.T
    g = np.asarray(ln_g, np.float32)
    bb = np.asarray(ln_b, np.float32)
    if not (np.all(g == 1.0) and np.all(bb == 0.0)):
        out_full = out_full * g[None, None, :] + bb[None, None, :]
    rs = avg.sum(axis=-1)  # [B, S], ~1 everywhere
    rs = rs - rs.max(axis=-1, keepdims=True)
    e = np.exp(rs)
    guilt = (e / e.sum(axis=-1, keepdims=True)).astype(np.float32)
    return out_full, avg, guilt


def _run(inputs, trace=False, trace_kwargs=None):
    nc = _get_nc()
    emb = np.asarray(inputs["embeddings"], np.float32)
    ins = _prep_core_inputs(
        emb,
        np.asarray(inputs["Wq"], np.float32),
        np.asarray(inputs["Wk"], np.float32),
        np.asarray(inputs["Wv"], np.float32),
        np.asarray(inputs["bias_scalars"], np.float32),
        np.asarray(inputs["Wo_w"], np.float32),
        np.asarray(inputs["Wo_b"], np.float32),
        np.asarray(inputs["ln_g"], np.float32),
        np.asarray(inputs["ln_b"], np.float32),
    )
    res = run_bass_kernel_spmd(
        nc, ins, core_ids=list(range(8)), trace=trace, **(trace_kwargs or {})
    )
    outs = _postprocess(
        res.results, np.asarray(inputs["ln_g"]), np.asarray(inputs["ln_b"])
    )
    return outs, res


def kernel(**inputs):
    outs, _ = _run(inputs, trace=False)
    return outs


# revision 17
# speedup vs baseline: 1.0396x; 1.0396x over previous
"""Trainium2 Bass kernel for nn_MultiHeadLegalAttention (B=4, S=2048, D=1024, H=4).

Sharding: 8 cores = (batch b in 0..4) x (query-half in 0..2). Each core
computes the 4-head "legal" attention for its 1024 query rows against the
full 2048-key context of its batch, plus the output projection, residual
and layernorm for those rows. Per-head exp-score tiles are shipped raw
(f16) and the host normalizes them into average_weights.

Device layout (per core, [partition, free]):
  xT      [D, S]   embeddings[b].T                (rhs for K/V projections)
  qT/kT   [e, s]   per-head projected Q/K, transposed (e on partitions)
  scoresT [j, i]   key j on partitions, query i on free axis
  softmax denominator via ones-matmul over partitions; attention output
  attT[e, i] accumulated in PSUM over key chunks, scaled by 1/colsum,
  concatenated to concatT[c, i]; out[i, f] = concatT.T @ Wo_w.T + residual.

Matmul dtypes: float32r (full fp32 operands, relaxed multiply, ~1.5e-4)
for the score path; f16 for the post-softmax path (P in [0, e^~6]).
"""

import numpy as np

import concourse.mybir as mybir
from concourse import bacc
from concourse.tile import TileContext
from concourse.bass_utils import run_bass_kernel_spmd

B, S, D, H, HD = 4, 2048, 1024, 4, 256
SQ = S // 2           # queries per core
NKC = S // 128        # key chunks of 128
NQ2 = SQ // 512       # 512-wide free-dim slices of the query range
LN_EPS = 1e-5

F32 = mybir.dt.float32
F32R = mybir.dt.float32r
F16 = mybir.dt.float16
ALU = mybir.AluOpType
ACTF = mybir.ActivationFunctionType

_CACHE = {}


def _build():
    nc = bacc.Bacc("TRN2", target_bir_lowering=False, debug=False)

    # xT columns are host-permuted so this core's 1024 query columns come
    # first; key chunk kc covers permuted positions [kc*128, kc*128+128) and
    # all j-indexed side inputs (jv/mj/vj/bias2) use the same permutation.
    d_xT = nc.dram_tensor("xT", [D, S], F32R, kind="ExternalInput")
    d_wqT = nc.dram_tensor("wqT", [H, 128, 2, HD], F32R, kind="ExternalInput")
    d_wkT = nc.dram_tensor("wkT", [H, 128, 2, HD], F32R, kind="ExternalInput")
    d_wvT = nc.dram_tensor("wvT", [H, 128, 2, HD], F32R, kind="ExternalInput")
    d_b2 = nc.dram_tensor("bias2", [NKC, 128, SQ], F16, kind="ExternalInput")
    d_ibc = nc.dram_tensor("ibc", [128, SQ], F16, kind="ExternalInput")
    d_mibc = nc.dram_tensor("mibc", [128, SQ], F32, kind="ExternalInput")
    d_vibc = nc.dram_tensor("vibc", [128, SQ], F32, kind="ExternalInput")
    d_jv = nc.dram_tensor("jv", [128, NKC], F32, kind="ExternalInput")
    d_mj = nc.dram_tensor("mj", [128, NKC], F32, kind="ExternalInput")
    d_vj = nc.dram_tensor("vj", [128, NKC], F32, kind="ExternalInput")
    d_thr = nc.dram_tensor("thr", [128, 1], F32, kind="ExternalInput")
    d_bsc = nc.dram_tensor("bsc", [128, H], F32, kind="ExternalInput")
    d_emb = nc.dram_tensor("embr", [SQ, D], F32, kind="ExternalInput")
    d_woT = nc.dram_tensor("woT", [D, D], F16, kind="ExternalInput")

    d_out = nc.dram_tensor("out", [SQ, D], F32, kind="ExternalOutput")
    d_expw = nc.dram_tensor("expw", [H, NKC, 128, SQ], F16, kind="ExternalOutput")

    with TileContext(nc) as tc:
        with (
            tc.tile_pool(name="const", bufs=1) as pc,
            tc.tile_pool(name="wts", bufs=2) as pw,
            tc.tile_pool(name="xin", bufs=2) as px,
            tc.tile_pool(name="pq", bufs=1) as pq,
            tc.tile_pool(name="pkv", bufs=2) as pkv,
            tc.tile_pool(name="pexp", bufs=5) as pexp,
            tc.tile_pool(name="pmask", bufs=3) as pmask,
            tc.tile_pool(name="big", bufs=1) as pbig,
            tc.tile_pool(name="pb2", bufs=2) as pb2,
            tc.tile_pool(name="psb", bufs=4) as psb,
            tc.tile_pool(name="prr", bufs=2) as prr,
            tc.tile_pool(name="fin", bufs=2) as pfin,
            tc.tile_pool(name="ps_s", bufs=3, space="PSUM") as ps_s,
            tc.tile_pool(name="ps_a", bufs=1, space="PSUM") as ps_a,
            tc.tile_pool(name="ps_c", bufs=1, space="PSUM") as ps_c,
        ):
            def emit_kv_dma(h):
                """input DMAs for head h's K/V projections."""
                t_x = []
                for dc in range(2):
                    tx = px.tile([128, S], F32R, tag="x", name="x")
                    nc.sync.dma_start(
                        out=tx[:],
                        in_=d_xT.ap()[(2 * h + dc) * 128:(2 * h + dc + 1) * 128, :],
                    )
                    t_x.append(tx)
                t_wk = pw.tile([128, 2, HD], F32R, tag="wk", name="wk")
                nc.sync.dma_start(out=t_wk[:], in_=d_wkT.ap()[h])
                t_wv = pw.tile([128, 2, HD], F32R, tag="wv", name="wv")
                nc.sync.dma_start(out=t_wv[:], in_=d_wvT.ap()[h])
                return t_x, t_wk, t_wv

            def emit_kv_mm(h, t_x, t_wk, t_wv):
                t_kT = pkv.tile([128, 2, S], F32R, tag="kT", name="kT")
                for ec in range(2):
                    for nk in range(4):
                        psk = ps_s.tile([128, 512], F32, tag="pss", name="psk")
                        for dc in range(2):
                            nc.tensor.matmul(
                                psk[:],
                                t_wk[:, dc, ec * 128:(ec + 1) * 128],
                                t_x[dc][:, nk * 512:(nk + 1) * 512],
                                start=(dc == 0), stop=(dc == 1),
                            )
                        nc.scalar.copy(t_kT[:, ec, nk * 512:(nk + 1) * 512], psk[:])
                t_v = pkv.tile([128, NKC, HD], F16, tag="v", name="v")
                for kc in range(NKC):
                    psv = ps_s.tile([128, 512], F32, tag="pss", name="psv")
                    for dc in range(2):
                        nc.tensor.matmul(
                            psv[:, 0:HD],
                            t_x[dc][:, kc * 128:(kc + 1) * 128],
                            t_wv[:, dc, :],
                            start=(dc == 0), stop=(dc == 1),
                        )
                    nc.scalar.copy(t_v[:, kc, :], psv[:, 0:HD])
                return t_kT, t_v

            def emit_q(h, t_x):
                """Q^T for head h (query columns are xT cols [0, SQ))."""
                t_wq = pw.tile([128, 2, HD], F32R, tag="wq", name="wq")
                nc.sync.dma_start(out=t_wq[:], in_=d_wqT.ap()[h])
                t_qT = pq.tile([128, 2, SQ], F32R, tag="qT", name="qT")
                for ec in range(2):
                    for nq in range(NQ2):
                        psq = ps_s.tile([128, 512], F32, tag="pss", name="psq")
                        for dc in range(2):
                            nc.tensor.matmul(
                                psq[:],
                                t_wq[:, dc, ec * 128:(ec + 1) * 128],
                                t_x[dc][:, nq * 512:(nq + 1) * 512],
                                start=(dc == 0), stop=(dc == 1),
                            )
                        nc.scalar.copy(t_qT[:, ec, nq * 512:(nq + 1) * 512], psq[:])
                return t_qT

            # ================= heads =================
            # head-0 input DMAs go first so the PE can start ASAP
            _kv = emit_kv_dma(0)
            t_x = _kv[0]

            # --- constants (small DMAs, queued behind head-0 inputs) ---
            t_ibc = pc.tile([128, SQ], F16)
            nc.sync.dma_start(out=t_ibc[:], in_=d_ibc.ap())
            t_mibc = pc.tile([128, SQ], F32)
            nc.sync.dma_start(out=t_mibc[:], in_=d_mibc.ap())
            t_vibc = pc.tile([128, SQ], F32)
            nc.sync.dma_start(out=t_vibc[:], in_=d_vibc.ap())
            t_jv = pc.tile([128, NKC], F32)
            nc.sync.dma_start(out=t_jv[:], in_=d_jv.ap())
            t_mj = pc.tile([128, NKC], F32)
            nc.sync.dma_start(out=t_mj[:], in_=d_mj.ap())
            t_vj = pc.tile([128, NKC], F32)
            nc.sync.dma_start(out=t_vj[:], in_=d_vj.ap())
            t_thr = pc.tile([128, 1], F32)
            nc.sync.dma_start(out=t_thr[:], in_=d_thr.ap())
            t_bsc = pc.tile([128, H], F32)
            nc.sync.dma_start(out=t_bsc[:], in_=d_bsc.ap())
            t_ones = pc.tile([128, 2], F16)
            nc.vector.memset(t_ones[:], 1.0)
            t_eps = pc.tile([128, 1], F32)
            nc.vector.memset(t_eps[:], LN_EPS)
            t_wo = pbig.tile([128, 8, D], F16, tag="wo")
            for cc in range(8):
                nc.sync.dma_start(
                    out=t_wo[:, cc, :], in_=d_woT.ap()[cc * 128:(cc + 1) * 128, :]
                )
            t_cat = pbig.tile([128, 8, SQ], F16, tag="cat")

            t_kT, t_v = emit_kv_mm(0, *_kv)
            t_qT = emit_q(0, t_x)
            nxt = None
            for h in range(H):
                att = [ps_a.tile([128, 512], F32, tag=f"att{i}", name=f"att{i}") for i in range(4)]
                csum_all = ps_c.tile([34, 512], F32, tag="csa", name="csa")

                def emit_score_chunk(kc):
                    pss = [ps_s.tile([128, 512], F32, tag="pss", name=f"pss{i}") for i in range(NQ2)]
                    for ec in range(2):
                        for nq in range(NQ2):
                            nc.tensor.matmul(
                                pss[nq][:],
                                t_kT[:, ec, kc * 128:(kc + 1) * 128],
                                t_qT[:, ec, nq * 512:(nq + 1) * 512],
                                start=(ec == 0), stop=(ec == 1),
                            )
                    # bias, on [j, i] tiles: j = perm[kc*128 + p], i = q0 + col
                    if h == 0:
                        msk = pmask.tile([128, SQ], F16, tag="msk", name="msk")
                        nc.vector.tensor_scalar(
                            out=msk[:], in0=t_ibc[:],
                            scalar1=t_jv[:, kc:kc + 1], scalar2=None, op0=ALU.is_lt,
                        )
                    elif h == 1:
                        msk = pmask.tile([128, SQ], F16, tag="msk", name="msk")
                        nc.vector.tensor_scalar(
                            out=msk[:], in0=t_mibc[:],
                            scalar1=t_mj[:, kc:kc + 1], scalar2=t_thr[:],
                            op0=ALU.mult, op1=ALU.is_gt,
                        )
                    elif h == 3:
                        msk = pmask.tile([128, SQ], F16, tag="msk", name="msk")
                        nc.vector.tensor_scalar(
                            out=msk[:], in0=t_vibc[:],
                            scalar1=t_vj[:, kc:kc + 1], scalar2=0.5,
                            op0=ALU.mult, op1=ALU.is_gt,
                        )
                    else:  # h == 2: temporal proximity, streamed from DRAM
                        msk = pb2.tile([128, SQ], F16, tag="b2", name="b2")
                        nc.sync.dma_start(out=msk[:], in_=d_b2.ap()[kc])

                    t_e = pexp.tile([128, SQ], F16, tag="expP", name="expP")
                    for nq in range(NQ2):
                        sl = slice(nq * 512, (nq + 1) * 512)
                        t_sb = psb.tile([128, 512], F32, tag="sb", name="sb")
                        if h == 2:
                            nc.vector.tensor_tensor(
                                out=t_sb[:], in0=pss[nq][:], in1=msk[:, sl],
                                op=ALU.add,
                            )
                        else:
                            nc.vector.scalar_tensor_tensor(
                                out=t_sb[:], in0=msk[:, sl],
                                scalar=t_bsc[:, h:h + 1], in1=pss[nq][:],
                                op0=ALU.mult, op1=ALU.add,
                            )
                        nc.scalar.activation(out=t_e[:, sl], in_=t_sb[:], func=ACTF.Exp)
                    return t_e

                def emit_consume_chunk(kc, t_e):
                    for nq in range(NQ2):
                        nc.tensor.matmul(
                            csum_all[nq * 32:nq * 32 + 2, :], t_ones[:],
                            t_e[:, nq * 512:(nq + 1) * 512],
                            start=(kc == 0), stop=(kc == NKC - 1),
                        )
                    for ec in range(2):
                        for nq in range(NQ2):
                            nc.tensor.matmul(
                                att[2 * ec + nq][:],
                                t_v[:, kc, ec * 128:(ec + 1) * 128],
                                t_e[:, nq * 512:(nq + 1) * 512],
                                start=(kc == 0), stop=(kc == NKC - 1),
                            )
                    nc.sync.dma_start(out=d_expw.ap()[h, kc], in_=t_e[:])

                pending = []
                for kc in range(NKC):
                    t_e = emit_score_chunk(kc)
                    pending.append((kc, t_e))
                    if len(pending) > 1:
                        emit_consume_chunk(*pending.pop(0))
                    if kc == 0 and h + 1 < H:
                        nxt_in = emit_kv_dma(h + 1)
                while pending:
                    emit_consume_chunk(*pending.pop(0))

                # ---- normalize attT into concatT ----
                t_cs = prr.tile([1, SQ], F32, tag="cs")
                for nq in range(NQ2):
                    nc.scalar.copy(t_cs[0:1, nq * 512:(nq + 1) * 512],
                                   csum_all[nq * 32:nq * 32 + 1, :])
                # reciprocal is iterative (8 cyc/elem): spread the 1024 sums
                # across all 128 partitions via DMA reshape, recip, reshape back
                t_r8 = prr.tile([128, 8], F32, tag="r8")
                nc.gpsimd.dma_start(out=t_r8[:], in_=t_cs[0:1, :])
                nc.vector.reciprocal(t_r8[:], t_r8[:])
                t_cr = prr.tile([1, SQ], F32, tag="cr")
                nc.gpsimd.dma_start(out=t_cr[0:1, :], in_=t_r8[:])
                t_rbc = prr.tile([128, SQ], F32, tag="rbc")
                nc.gpsimd.partition_broadcast(t_rbc[:], t_cr[0:1, :])
                for ec in range(2):
                    for nq in range(NQ2):
                        sl = slice(nq * 512, (nq + 1) * 512)
                        nc.vector.tensor_tensor(
                            out=t_cat[:, 2 * h + ec, sl],
                            in0=att[2 * ec + nq][:], in1=t_rbc[:, sl], op=ALU.mult,
                        )
                if h + 1 < H:
                    t_kT, t_v = emit_kv_mm(h + 1, *nxt_in)
                    t_qT = emit_q(h + 1, nxt_in[0])

            # ================= output projection + layernorm =================
            for m in range(8):
                t_emb = pfin.tile([128, D], F32, tag="emb")
                nc.sync.dma_start(out=t_emb[:], in_=d_emb.ap()[m * 128:(m + 1) * 128, :])
                t_y = pfin.tile([128, D], F32, tag="y")
                for nf in range(2):
                    pso = ps_s.tile([128, 512], F32, tag="pss", name="pso")
                    for cc in range(8):
                        nc.tensor.matmul(
                            pso[:],
                            t_cat[:, cc, m * 128:(m + 1) * 128],
                            t_wo[:, cc, nf * 512:(nf + 1) * 512],
                            start=(cc == 0), stop=(cc == 7),
                        )
                    nc.vector.tensor_tensor(
                        out=t_y[:, nf * 512:(nf + 1) * 512], in0=pso[:],
                        in1=t_emb[:, nf * 512:(nf + 1) * 512], op=ALU.add,
                    )
                t_st = pfin.tile([128, 2, 6], F32, tag="st")
                for g in range(2):
                    nc.vector.bn_stats(out=t_st[:, g, :], in_=t_y[:, g * 512:(g + 1) * 512])
                t_mv = pfin.tile([128, 2], F32, tag="mv")
                nc.vector.bn_aggr(out=t_mv[:], in_=t_st[:])
                t_sd = pfin.tile([128, 1], F32, tag="sd")
                nc.scalar.activation(
                    out=t_sd[:], in_=t_mv[:, 1:2], func=ACTF.Sqrt, bias=t_eps[:], scale=1.0
                )
                nc.vector.reciprocal(t_sd[:], t_sd[:])
                t_ms = pfin.tile([128, 1], F32, tag="ms")
                nc.vector.tensor_scalar(
                    out=t_ms[:], in0=t_mv[:, 0:1], scalar1=t_sd[:], scalar2=None,
                    op0=ALU.mult,
                )
                t_o = pfin.tile([128, D], F32, tag="o")
                nc.vector.tensor_scalar(
                    out=t_o[:], in0=t_y[:], scalar1=t_sd[:], scalar2=t_ms[:],
                    op0=ALU.mult, op1=ALU.subtract,
                )
                nc.sync.dma_start(out=d_out.ap()[m * 128:(m + 1) * 128, :], in_=t_o[:])

    nc.compile()
    return nc


def _get_nc():
    if "nc" not in _CACHE:
        _CACHE["nc"] = _build()
    return _CACHE["nc"]


def _prep_core_inputs(emb, Wq, Wk, Wv, bias_scalars, Wo_w, Wo_b, ln_g, ln_b):
    """Host-side prep: returns list of 8 per-core input dicts."""
    scale = np.float32(1.0 / np.sqrt(HD))
    def _wlayout(w):
        # [H, e, d] -> wT[h, d, e] -> [H, p, dc, e] with d = dc*128 + p
        wt = w.transpose(0, 2, 1).reshape(H, 2, 128, HD).transpose(0, 2, 1, 3)
        return np.ascontiguousarray(wt).astype(np.float32)
    wqT = _wlayout(Wq * scale)
    wkT = _wlayout(Wk)
    wvT = _wlayout(Wv)
    woT = np.ascontiguousarray(Wo_w.T).astype(np.float16)

    mag1 = np.sqrt((emb[:, :, HD:2 * HD].astype(np.float32) ** 2).sum(-1))  # [B, S]
    ssum = mag1.astype(np.float64).sum(axis=1)
    thr_v = np.float32((ssum ** 2).sum() / (B * S * S))
    v3 = emb[:, :, D - 1].astype(np.float32)  # [B, S]

    thr = np.full((128, 1), thr_v, np.float32)
    bsc = np.ascontiguousarray(
        np.broadcast_to(np.asarray(bias_scalars, np.float32), (128, H))
    )

    ins = []
    for c in range(8):
        b, half = divmod(c, 2)
        q0 = half * SQ
        xb = emb[b]
        # key-column permutation: this core's query block first
        perm = (np.arange(S) + q0) % S
        xT = np.ascontiguousarray(xb.T.astype(np.float32)[:, perm])
        iv = (q0 + np.arange(SQ, dtype=np.float32))
        ibc = np.ascontiguousarray(np.broadcast_to(iv.astype(np.float16), (128, SQ)))
        mibc = np.ascontiguousarray(np.broadcast_to(mag1[b, q0:q0 + SQ], (128, SQ)))
        vibc = np.ascontiguousarray(np.broadcast_to(v3[b, q0:q0 + SQ], (128, SQ)))
        jvp = perm.astype(np.float32)
        jv_c = np.ascontiguousarray(jvp.reshape(NKC, 128).T)
        mj = np.ascontiguousarray(mag1[b][perm].reshape(NKC, 128).T)
        vj = np.ascontiguousarray(v3[b][perm].reshape(NKC, 128).T)
        jj = perm.astype(np.float64)[:, None]
        ii = (q0 + np.arange(SQ, dtype=np.float64))[None, :]
        b2 = (np.float64(bias_scalars[2]) * np.exp(-np.abs(ii - jj) / S)).astype(
            np.float16
        ).reshape(NKC, 128, SQ)
        embr = (xb[q0:q0 + SQ, :] + np.asarray(Wo_b, np.float32)[None, :]).astype(
            np.float32
        )
        ins.append({
            "xT": xT, "wqT": wqT, "wkT": wkT, "wvT": wvT,
            "bias2": np.ascontiguousarray(b2), "ibc": ibc, "mibc": mibc,
            "vibc": vibc, "jv": jv_c, "mj": mj, "vj": vj, "thr": thr, "bsc": bsc,
            "embr": embr, "woT": woT,
        })
    return ins


def _postprocess(results, ln_g, ln_b):
    out_full = np.empty((B, S, D), np.float32)
    avg = np.empty((B, S, S), np.float32)
    for c in range(8):
        b, half = divmod(c, 2)
        q0 = half * SQ
        out_full[b, q0:q0 + SQ] = results[c]["out"]
        ew = results[c]["expw"].reshape(H, S, SQ).astype(np.float32)  # [h, j_perm, i]
        cs = ew.sum(axis=1)  # [h, i]
        w = np.einsum("hji,hi->ji", ew, (1.0 / cs).astype(np.float32))
        w = np.roll(w, q0, axis=0)  # undo key-column permutation
        avg[b, q0:q0 + SQ, :] = 0.25 * w.T
    g = np.asarray(ln_g, np.float32)
    bb = np.asarray(ln_b, np.float32)
    if not (np.all(g == 1.0) and np.all(bb == 0.0)):
        out_full = out_full * g[None, None, :] + bb[None, None, :]
    rs = avg.sum(axis=-1)  # [B, S], ~1 everywhere
    rs = rs - rs.max(axis=-1, keepdims=True)
    e = np.exp(rs)
    guilt = (e / e.sum(axis=-1, keepdims=True)).astype(np.float32)
    return out_full, avg, guilt


def _run(inputs, trace=False, trace_kwargs=None):
    nc = _get_nc()
    emb = np.asarray(inputs["embeddings"], np.float32)
    ins = _prep_core_inputs(
        emb,
        np.asarray(inputs["Wq"], np.float32),
        np.asarray(inputs["Wk"], np.float32),
        np.asarray(inputs["Wv"], np.float32),
        np.asarray(inputs["bias_scalars"], np.float32),
        np.asarray(inputs["Wo_w"], np.float32),
        np.asarray(inputs["Wo_b"], np.float32),
        np.asarray(inputs["ln_g"], np.float32),
        np.asarray(inputs["ln_b"], np.float32),
    )
    res = run_bass_kernel_spmd(
        nc, ins, core_ids=list(range(8)), trace=trace, **(trace_kwargs or {})
    )
    outs = _postprocess(
        res.results, np.asarray(inputs["ln_g"]), np.asarray(inputs["ln_b"])
    )
    return outs, res


def kernel(**inputs):
    outs, _ = _run(inputs, trace=False)
    return outs
